# revision 41
# baseline (speedup 1.0000x reference)
"""Bahdanau (additive) attention kernel for Trainium2, SPMD over 8 NeuronCores.

Math:  q = query @ Wq ; k = value @ Wv
       scores[b,i,j] = sum_d scale[d] * tanh(q[b,i,d] + k[b,j,d])
       attn = softmax(scores, axis=-1) ; out = attn @ value
Returns (out, attn) like the reference.

Strategy: tanh(z) ~= sum_{r=1..15} a_r sin(r*om1*z) on z in [-8.8, 8.8]
(truncated Fourier sine series, weighted LS fit).  sin(r om1 (x+y)) =
sin(r om1 x)cos(r om1 y) + cos(r om1 x)sin(r om1 y) is separable, so the
score tensor becomes one PE matmul with contraction dim 2*R*D, with
operands cast to fp16 (fp32 matmul streams 4x slower).  R=15, L=9.7
plus fp16 rounding gives ~1.1e-3 end-to-end rel err (attn absmax 4e-3).

Harmonic generation (ACT Sin table only covers [-pi, pi]):
 - base sin/cos at HALF frequency om1/2 via ACT Sin (args always in range),
   then angle-doubling to (sin om1 x, cos om1 x);
 - even r: sin2m = (sin_m*2)*cos_m as one fused scalar_tensor_tensor on DVE,
   cos2m = 1 - 2*Square(sin_m) with Square + affine Copy on ScalarE;
 - odd r: Chebyshev 3-term recurrence (2 tensor_tensors per map) on DVE.
The series coefficient a_r and channel weight scale[d] fold into the
per-r fp16 lhsT cast (one tensor_scalar per map with per-partition scalar).

Softmax runs without the row-max pass (scores are bounded ~|s|<80, far from
the fp32 exp-overflow threshold 88.7) and the row-sum accumulates inside the
Exp instruction (accum_out), so the probabilities need one reciprocal and one
tensor_scalar only.

Overlap details: base-map Sins read the projection PSUM tiles directly
(no SBUF staging); input-transpose PSUM->SBUF copies run on VectorE (idle
during the front-end) so ScalarE reaches the base maps sooner; the attn
transpose uses the unnormalized exp output (available before the reciprocal)
with row-normalization folded into the out PSUM->SBUF tensor_scalar.

Engine balance (cost model): DVE ~81us, ACT ~70us, PE ~42us -> ~99us/core.
Sharding: 8 cores = (batch b) x (query-half ih); no collectives.
"""
import numpy as np

B, TQ, TK, D = 4, 512, 512, 256
N_CORES = 8
TQ_C = TQ // 2          # 256 query rows per core
P = 128                 # partitions

OM1 = 0.3141592653589793
COEFS = [1.241760095, -0.03380200667, 0.3445502297, -0.04630709654,
         0.1522961562, -0.04060338012, 0.07448292054, -0.02840172445,
         0.03714179193, -0.01691867202, 0.01784833865, -0.00862484866,
         0.007807306274, -0.003595989083, 0.002843761099, -0.001104257103,
         0.0007395428243]
R = len(COEFS)

_CACHE = {}


def _build_program(repeats=1):
    import concourse.bass as bass
    import concourse.mybir as mybir
    import concourse.tile as tile
    from concourse import bacc
    from concourse.masks import make_identity

    f32 = mybir.dt.float32
    f16 = mybir.dt.float16
    AF = mybir.ActivationFunctionType
    ALU = mybir.AluOpType
    HALF_PI = float(np.pi / 2)

    nc = bacc.Bacc("TRN2", target_bir_lowering=False, debug=False,
                   num_devices=N_CORES)

    query = nc.dram_tensor("query", [TQ_C, D], f32, kind="ExternalInput").ap()
    value = nc.dram_tensor("value", [TK, D], f32, kind="ExternalInput").ap()
    wq = nc.dram_tensor("wq", [D, D], f32, kind="ExternalInput").ap()
    wv = nc.dram_tensor("wv", [D, D], f32, kind="ExternalInput").ap()
    scale = nc.dram_tensor("scale", [D], f32, kind="ExternalInput").ap()
    out = nc.dram_tensor("out", [TQ_C, D], f32, kind="ExternalOutput").ap()
    attn = nc.dram_tensor("attn", [TQ_C, TK], f32, kind="ExternalOutput").ap()

    with tile.TileContext(nc) as tc:
      for _rep in range(repeats):
        with (
            tc.tile_pool(name="consts", bufs=1) as consts,
            tc.tile_pool(name="io", bufs=1) as io,
            tc.tile_pool(name="qmaps", bufs=9) as qmaps,
            tc.tile_pool(name="kmaps", bufs=9) as kmaps,
            tc.tile_pool(name="qtmp", bufs=2) as qtmp,
            tc.tile_pool(name="ktmp", bufs=2) as ktmp,
            tc.tile_pool(name="wmaps", bufs=4) as wmaps,
            tc.tile_pool(name="soft", bufs=2) as soft,
            tc.tile_pool(name="psum_t", bufs=2, space="PSUM") as psum_t,
            tc.tile_pool(name="psum_s", bufs=2, space="PSUM") as psum_s,
            tc.tile_pool(name="psum_o", bufs=2, space="PSUM") as psum_o,
        ):
            # ---------------- loads ----------------
            ident = consts.tile([P, P], f32)
            make_identity(nc, ident)

            # scale first (tiny, own queue) so the wcol precompute can start
            s_sb = consts.tile([P, 2], f32)
            nc.gpsimd.dma_start(out=s_sb, in_=scale.rearrange("(t p) -> p t", p=P))
            wcols = []
            for r in range(1, R + 1):
                w = consts.tile([P, 2], f32, name=f"wcol{r}")
                nc.vector.tensor_scalar_mul(w, s_sb, float(COEFS[r - 1]))
                wcols.append(w)

            q_sb = []          # query [i, in] as 2 row tiles [128, 256]
            for m in range(2):
                t = io.tile([P, D], f32, tag=f"q_sb{m}", name=f"q_sb{m}")
                nc.sync.dma_start(out=t, in_=query[bass.ts(m, P), :])
                q_sb.append(t)
            v_sb = []          # value [j, in] as 4 row tiles [128, 256]
            for jc in range(4):
                t = io.tile([P, D], f32, tag=f"v_sb{jc}", name=f"v_sb{jc}")
                nc.sync.dma_start(out=t, in_=value[bass.ts(jc, P), :])
                v_sb.append(t)
            wq_sb = []         # Wq [in, d] as 2 in-chunk tiles [128, 256]
            wv_sb = []
            for ic in range(2):
                t = io.tile([P, D], f32, tag=f"wq_sb{ic}", name=f"wq_sb{ic}")
                nc.sync.dma_start(out=t, in_=wq[bass.ts(ic, P), :])
                wq_sb.append(t)
                t = io.tile([P, D], f32, tag=f"wv_sb{ic}", name=f"wv_sb{ic}")
                nc.sync.dma_start(out=t, in_=wv[bass.ts(ic, P), :])
                wv_sb.append(t)

            # ---------------- transposes of query / value ----------------
            # queryT [in, i]: 2 in-chunk tiles [128, 256]
            qT = [io.tile([P, 2 * P], f32, tag=f"qT{i}", name=f"qT{i}") for i in range(2)]
            for ic in range(2):
                for m in range(2):
                    pt = psum_t.tile([P, P], f32, tag="tr")
                    nc.tensor.transpose(pt, q_sb[m][:, bass.ts(ic, P)], ident)
                    nc.vector.tensor_copy(out=qT[ic][:, bass.ts(m, P)], in_=pt)
            # valueT [in, j]: 2 in-chunk tiles [128, 512]
            vT = [io.tile([P, TK], f32, tag=f"vT{i}", name=f"vT{i}") for i in range(2)]
            for ic in range(2):
                for jc in range(4):
                    pt = psum_t.tile([P, P], f32, tag="tr")
                    nc.tensor.transpose(pt, v_sb[jc][:, bass.ts(ic, P)], ident)
                    nc.vector.tensor_copy(out=vT[ic][:, bass.ts(jc, P)], in_=pt)

            # ---------------- projections (transposed layout) ----------------
            # Projections stay in PSUM; the base-map Sin ops read them there
            # (ScalarE is closer to PSUM) -- no SBUF staging copy.
            qp_ps = []
            for t in range(2):
                pp = psum_t.tile([P, TQ_C], f32, tag="proj", bufs=4, name=f"qp_ps{t}")
                for ic in range(2):
                    nc.tensor.matmul(pp, wq_sb[ic][:, bass.ts(t, P)], qT[ic],
                                     start=(ic == 0), stop=(ic == 1))
                qp_ps.append(pp)
            kp_ps = []
            for t in range(2):
                pp = psum_t.tile([P, TK], f32, tag="proj", bufs=4, name=f"kp_ps{t}")
                for ic in range(2):
                    nc.tensor.matmul(pp, wv_sb[ic][:, bass.ts(t, P)], vT[ic],
                                     start=(ic == 0), stop=(ic == 1))
                kp_ps.append(pp)

            # ---------------- base maps ----------------
            FQ, FK = 2 * TQ_C, 2 * TK     # free sizes of packed q/k tiles

            halfpi = consts.tile([P, 1], f32)
            nc.vector.memset(halfpi, HALF_PI)

            # Base maps via HALF-frequency sin/cos (ACT Sin args stay well
            # inside [-pi, pi] for any om1 <= 0.6), then angle-doubling:
            #   sin(om1 x) = 2 sh ch,  cos(om1 x) = 1 - 2 sh^2
            shq = qtmp.tile([P, FQ], f32, tag="tq", name="shq")
            chq = qtmp.tile([P, FQ], f32, tag="tq2", name="chq")
            for t in range(2):
                nc.scalar.activation(shq[:, bass.ts(t, TQ_C)], qp_ps[t], AF.Sin,
                                     scale=OM1 / 2)
                nc.scalar.activation(chq[:, bass.ts(t, TQ_C)], qp_ps[t], AF.Sin,
                                     bias=halfpi, scale=OM1 / 2)
            sq1 = qmaps.tile([P, FQ], f32, tag="sq", name="sq1")
            nc.vector.scalar_tensor_tensor(sq1, shq, 2.0, chq,
                                           op0=ALU.mult, op1=ALU.mult)
            uq1 = qtmp.tile([P, FQ], f32, tag="tq", name="uq1")
            nc.scalar.square(uq1, shq)
            cq1 = qmaps.tile([P, FQ], f32, tag="cq", name="cq1")
            nc.vector.tensor_scalar(cq1, uq1, -2.0, 1.0, op0=ALU.mult, op1=ALU.add)
            c2q = consts.tile([P, FQ], f32)
            nc.vector.tensor_scalar_mul(c2q, cq1, 2.0)

            shk = ktmp.tile([P, FK], f32, tag="tk", name="shk")
            chk = ktmp.tile([P, FK], f32, tag="tk2", name="chk")
            for t in range(2):
                nc.scalar.activation(shk[:, bass.ts(t, TK)], kp_ps[t], AF.Sin,
                                     scale=OM1 / 2)
                nc.scalar.activation(chk[:, bass.ts(t, TK)], kp_ps[t], AF.Sin,
                                     bias=halfpi, scale=OM1 / 2)
            sk1 = kmaps.tile([P, FK], f32, tag="sk", name="sk1")
            nc.vector.scalar_tensor_tensor(sk1, shk, 2.0, chk,
                                           op0=ALU.mult, op1=ALU.mult)
            uk1 = ktmp.tile([P, FK], f32, tag="tk", name="uk1")
            nc.scalar.square(uk1, shk)
            ck1 = kmaps.tile([P, FK], f32, tag="ck", name="ck1")
            nc.vector.tensor_scalar(ck1, uk1, -2.0, 1.0, op0=ALU.mult, op1=ALU.add)
            c2k = consts.tile([P, FK], f32)
            nc.vector.tensor_scalar_mul(c2k, ck1, 2.0)


            # ---------------- main loop: ladder + score matmuls ----------------
            score_ps = [psum_s.tile([P, TK], f32, tag="score", name=f"score{m}") for m in range(2)]

            mm_cnt = [0, 0]
            mm_total = R * 2 * 2   # per m-tile: R harmonics x 2 t-chunks x 2 products
            sqm = {1: sq1}
            cqm = {1: cq1}
            skm = {1: sk1}
            ckm = {1: ck1}
            for r in range(1, R + 1):
                if r == 1:
                    pass
                elif r % 2 == 0:
                    m2 = r // 2
                    # sin(2m) = (sin(m)*2)*cos(m) in one fused op
                    sq_r = qmaps.tile([P, FQ], f32, tag="sq", name=f"sq{r}")
                    nc.vector.scalar_tensor_tensor(sq_r, sqm[m2], 2.0, cqm[m2],
                                                   op0=ALU.mult, op1=ALU.mult)
                    # cos(2m) = 1 - 2*sin(m)^2 : Square on ScalarE + 1 tensor_scalar
                    u = qtmp.tile([P, FQ], f32, tag="tq", name=f"usq{r}")
                    nc.scalar.square(u, sqm[m2])
                    cq_r = qmaps.tile([P, FQ], f32, tag="cq", name=f"cq{r}")
                    nc.scalar.activation(cq_r, u, AF.Copy, bias=1.0, scale=-2.0)
                    sk_r = kmaps.tile([P, FK], f32, tag="sk", name=f"sk{r}")
                    nc.vector.scalar_tensor_tensor(sk_r, skm[m2], 2.0, ckm[m2],
                                                   op0=ALU.mult, op1=ALU.mult)
                    uk = ktmp.tile([P, FK], f32, tag="tk", name=f"usk{r}")
                    nc.scalar.square(uk, skm[m2])
                    ck_r = kmaps.tile([P, FK], f32, tag="ck", name=f"ck{r}")
                    nc.scalar.activation(ck_r, uk, AF.Copy, bias=1.0, scale=-2.0)
                    sqm[r], cqm[r], skm[r], ckm[r] = sq_r, cq_r, sk_r, ck_r
                else:
                    # odd r: Chebyshev recurrence s_r = 2c1*s_{r-1} - s_{r-2}
                    tq1 = qtmp.tile([P, FQ], f32, tag="tq", name=f"tq1_{r}")
                    nc.vector.tensor_mul(tq1, c2q, sqm[r - 1])
                    sq_r = qmaps.tile([P, FQ], f32, tag="sq", name=f"sq{r}")
                    nc.vector.tensor_sub(sq_r, tq1, sqm[r - 2])
                    tq2 = qtmp.tile([P, FQ], f32, tag="tq", name=f"tq2_{r}")
                    nc.vector.tensor_mul(tq2, c2q, cqm[r - 1])
                    cq_r = qmaps.tile([P, FQ], f32, tag="cq", name=f"cq{r}")
                    nc.vector.tensor_sub(cq_r, tq2, cqm[r - 2])
                    tk1 = ktmp.tile([P, FK], f32, tag="tk", name=f"tk1_{r}")
                    nc.vector.tensor_mul(tk1, c2k, skm[r - 1])
                    sk_r = kmaps.tile([P, FK], f32, tag="sk", name=f"sk{r}")
                    nc.vector.tensor_sub(sk_r, tk1, skm[r - 2])
                    tk2 = ktmp.tile([P, FK], f32, tag="tk", name=f"tk2_{r}")
                    nc.vector.tensor_mul(tk2, c2k, ckm[r - 1])
                    ck_r = kmaps.tile([P, FK], f32, tag="ck", name=f"ck{r}")
                    nc.vector.tensor_sub(ck_r, tk2, ckm[r - 2])
                    sqm[r], cqm[r], skm[r], ckm[r] = sq_r, cq_r, sk_r, ck_r

                # weighted q-side copies: awsq = (a_r s_d) * sin_r, awcq = (a_r s_d) * cos_r
                # odd-r casts ride on ScalarE (it has headroom); even-r on VectorE
                awsq = wmaps.tile([P, FQ], f16, tag="awsq", name=f"awsq{r}")
                awcq = wmaps.tile([P, FQ], f16, tag="awcq", name=f"awcq{r}")
                for t in range(2):
                    sl = bass.ts(t, TQ_C)
                    if r > 1 and (r % 2 == 1 or r <= 6):
                        nc.scalar.activation(awsq[:, sl], sqm[r][:, sl], AF.Copy,
                                             scale=wcols[r - 1][:, t:t+1])
                        nc.scalar.activation(awcq[:, sl], cqm[r][:, sl], AF.Copy,
                                             scale=wcols[r - 1][:, t:t+1])
                    else:
                        nc.vector.tensor_scalar_mul(awsq[:, sl], sqm[r][:, sl],
                                                    wcols[r - 1][:, t:t+1])
                        nc.vector.tensor_scalar_mul(awcq[:, sl], cqm[r][:, sl],
                                                    wcols[r - 1][:, t:t+1])

                ck16 = wmaps.tile([P, FK], f16, tag="ck16", name=f"ck16_{r}")
                nc.scalar.copy(out=ck16, in_=ckm[r])
                sk16 = wmaps.tile([P, FK], f16, tag="sk16", name=f"sk16_{r}")
                nc.scalar.copy(out=sk16, in_=skm[r])

                for m in range(2):
                    for t in range(2):
                        lhs_sl = bass.ds(t * TQ_C + m * P, P)
                        rhs_sl = bass.ts(t, TK)
                        nc.tensor.matmul(score_ps[m], awsq[:, lhs_sl],
                                         ck16[:, rhs_sl],
                                         start=(mm_cnt[m] == 0), stop=False,
                                         skip_group_check=True)
                        nc.tensor.matmul(score_ps[m], awcq[:, lhs_sl],
                                         sk16[:, rhs_sl],
                                         start=False,
                                         stop=(mm_cnt[m] == mm_total - 1),
                                         skip_group_check=True)
                        mm_cnt[m] += 1

            # ---------------- softmax ----------------
            # softmax without row-max: scores bounded well below fp32 exp
            # overflow (88.7); row-sum accumulates inside the Exp (accum_out).
            p_list, rinv_list = [], []
            for m in range(2):
                p_sb = soft.tile([P, TK], f32, tag="p_sb", name=f"p_sb{m}")
                rsum = soft.tile([P, 1], f32, tag="rsum")
                nc.scalar.activation(p_sb, score_ps[m], AF.Exp, accum_out=rsum)
                rinv = soft.tile([P, 1], f32, tag="rinv", name=f"rinv{m}")
                nc.vector.reciprocal(rinv, rsum)
                at = soft.tile([P, TK], f32, tag="attn_sb")
                nc.vector.tensor_scalar_mul(at, p_sb, rinv)
                nc.sync.dma_start(out=attn[bass.ts(m, P), :], in_=at)
                p_list.append(p_sb)
                rinv_list.append(rinv)

            # ---------------- out = attn @ value ----------------
            # transpose attn -> attnT [j, i] as 4 j-chunk tiles [128, 256]
            attnT = [soft.tile([P, TQ_C], f32, tag=f"attnT{i}", name=f"attnT{i}") for i in range(4)]
            for jc in range(4):
                for m in range(2):
                    pt = psum_t.tile([P, P], f32, tag="tr")
                    nc.tensor.transpose(pt, p_list[m][:, bass.ts(jc, P)], ident)
                    nc.vector.tensor_copy(out=attnT[jc][:, bass.ts(m, P)], in_=pt)
            for m in range(2):
                po = psum_t.tile([P, D], f32, tag="proj", bufs=4, name=f"out_ps{m}")
                for jc in range(4):
                    nc.tensor.matmul(po, attnT[jc][:, bass.ts(m, P)], v_sb[jc],
                                     start=(jc == 0), stop=(jc == 3))
                o_sb = soft.tile([P, D], f32, tag="o_sb")
                nc.vector.tensor_scalar_mul(o_sb, po, rinv_list[m])
                nc.gpsimd.dma_start(out=out[bass.ts(m, P), :], in_=o_sb)

    nc.compile()
    return nc


def _get_program(repeats=1):
    key = f"nc{repeats}"
    if key not in _CACHE:
        _CACHE[key] = _build_program(repeats)
    return _CACHE[key]


def kernel(query, value, Wq, Wv, scale):
    from concourse.bass_utils import run_bass_kernel_spmd

    nc = _get_program()
    query = np.ascontiguousarray(query, dtype=np.float32)
    value = np.ascontiguousarray(value, dtype=np.float32)
    Wq = np.ascontiguousarray(Wq, dtype=np.float32)
    Wv = np.ascontiguousarray(Wv, dtype=np.float32)
    scale = np.ascontiguousarray(scale, dtype=np.float32)

    in_maps = []
    for c in range(N_CORES):
        b, ih = c // 2, c % 2
        in_maps.append({
            "query": query[b, ih * TQ_C:(ih + 1) * TQ_C, :],
            "value": value[b],
            "wq": Wq, "wv": Wv, "scale": scale,
        })
    res = run_bass_kernel_spmd(nc, in_maps, list(range(N_CORES)))

    out = np.empty((B, TQ, D), dtype=np.float32)
    attn = np.empty((B, TQ, TK), dtype=np.float32)
    for c in range(N_CORES):
        b, ih = c // 2, c % 2
        out[b, ih * TQ_C:(ih + 1) * TQ_C, :] = res.results[c]["out"]
        attn[b, ih * TQ_C:(ih + 1) * TQ_C, :] = res.results[c]["attn"]
    return out, attn


# revision 42
# speedup vs baseline: 1.0058x; 1.0058x over previous
"""Bahdanau (additive) attention kernel for Trainium2, SPMD over 8 NeuronCores.

Math:  q = query @ Wq ; k = value @ Wv
       scores[b,i,j] = sum_d scale[d] * tanh(q[b,i,d] + k[b,j,d])
       attn = softmax(scores, axis=-1) ; out = attn @ value
Returns (out, attn) like the reference.

Strategy: tanh(z) ~= sum_{r=1..15} a_r sin(r*om1*z) on z in [-8.8, 8.8]
(truncated Fourier sine series, weighted LS fit).  sin(r om1 (x+y)) =
sin(r om1 x)cos(r om1 y) + cos(r om1 x)sin(r om1 y) is separable, so the
score tensor becomes one PE matmul with contraction dim 2*R*D, with
operands cast to fp16 (fp32 matmul streams 4x slower).  R=15, L=9.7
plus fp16 rounding gives ~1.1e-3 end-to-end rel err (attn absmax 4e-3).

Harmonic generation (ACT Sin table only covers [-pi, pi]):
 - base sin/cos at HALF frequency om1/2 via ACT Sin (args always in range),
   then angle-doubling to (sin om1 x, cos om1 x);
 - even r: sin2m = (sin_m*2)*cos_m as one fused scalar_tensor_tensor on DVE,
   cos2m = 1 - 2*Square(sin_m) with Square + affine Copy on ScalarE;
 - odd r: Chebyshev 3-term recurrence (2 tensor_tensors per map) on DVE.
The series coefficient a_r and channel weight scale[d] fold into the
per-r fp16 lhsT cast (one tensor_scalar per map with per-partition scalar).

Softmax runs without the row-max pass (scores are bounded ~|s|<80, far from
the fp32 exp-overflow threshold 88.7) and the row-sum accumulates inside the
Exp instruction (accum_out), so the probabilities need one reciprocal and one
tensor_scalar only.

Overlap details: base-map Sins read the projection PSUM tiles directly
(no SBUF staging); input-transpose PSUM->SBUF copies run on VectorE (idle
during the front-end) so ScalarE reaches the base maps sooner; the attn
transpose uses the unnormalized exp output (available before the reciprocal)
with row-normalization folded into the out PSUM->SBUF tensor_scalar.

Engine balance (cost model): DVE ~81us, ACT ~70us, PE ~42us -> ~99us/core.
Sharding: 8 cores = (batch b) x (query-half ih); no collectives.
"""
import numpy as np

B, TQ, TK, D = 4, 512, 512, 256
N_CORES = 8
TQ_C = TQ // 2          # 256 query rows per core
P = 128                 # partitions

OM1 = 0.3141592653589793
COEFS = [1.241760095, -0.03380200667, 0.3445502297, -0.04630709654,
         0.1522961562, -0.04060338012, 0.07448292054, -0.02840172445,
         0.03714179193, -0.01691867202, 0.01784833865, -0.00862484866,
         0.007807306274, -0.003595989083, 0.002843761099, -0.001104257103,
         0.0007395428243]
R = len(COEFS)

_CACHE = {}


def _build_program(repeats=1):
    import concourse.bass as bass
    import concourse.mybir as mybir
    import concourse.tile as tile
    from concourse import bacc
    from concourse.masks import make_identity

    f32 = mybir.dt.float32
    f16 = mybir.dt.float16
    AF = mybir.ActivationFunctionType
    ALU = mybir.AluOpType
    HALF_PI = float(np.pi / 2)

    nc = bacc.Bacc("TRN2", target_bir_lowering=False, debug=False,
                   num_devices=N_CORES)

    query = nc.dram_tensor("query", [TQ_C, D], f32, kind="ExternalInput").ap()
    value = nc.dram_tensor("value", [TK, D], f32, kind="ExternalInput").ap()
    wq = nc.dram_tensor("wq", [D, D], f32, kind="ExternalInput").ap()
    wv = nc.dram_tensor("wv", [D, D], f32, kind="ExternalInput").ap()
    scale = nc.dram_tensor("scale", [D], f32, kind="ExternalInput").ap()
    out = nc.dram_tensor("out", [TQ_C, D], f32, kind="ExternalOutput").ap()
    attn = nc.dram_tensor("attn", [TQ_C, TK], f32, kind="ExternalOutput").ap()

    with tile.TileContext(nc) as tc:
      for _rep in range(repeats):
        with (
            tc.tile_pool(name="consts", bufs=1) as consts,
            tc.tile_pool(name="io", bufs=1) as io,
            tc.tile_pool(name="qmaps", bufs=9) as qmaps,
            tc.tile_pool(name="kmaps", bufs=9) as kmaps,
            tc.tile_pool(name="qtmp", bufs=2) as qtmp,
            tc.tile_pool(name="ktmp", bufs=2) as ktmp,
            tc.tile_pool(name="wmaps", bufs=4) as wmaps,
            tc.tile_pool(name="soft", bufs=2) as soft,
            tc.tile_pool(name="psum_t", bufs=2, space="PSUM") as psum_t,
            tc.tile_pool(name="psum_s", bufs=2, space="PSUM") as psum_s,
            tc.tile_pool(name="psum_o", bufs=2, space="PSUM") as psum_o,
        ):
            # ---------------- loads ----------------
            ident = consts.tile([P, P], f32)
            make_identity(nc, ident)

            # scale first (tiny, own queue) so the wcol precompute can start
            s_sb = consts.tile([P, 2], f32)
            nc.gpsimd.dma_start(out=s_sb, in_=scale.rearrange("(t p) -> p t", p=P))
            wcols = []
            for r in range(1, R + 1):
                w = consts.tile([P, 2], f32, name=f"wcol{r}")
                nc.vector.tensor_scalar_mul(w, s_sb, float(COEFS[r - 1]))
                wcols.append(w)

            q_sb = []          # query [i, in] as 2 row tiles [128, 256]
            for m in range(2):
                t = io.tile([P, D], f32, tag=f"q_sb{m}", name=f"q_sb{m}")
                nc.sync.dma_start(out=t, in_=query[bass.ts(m, P), :])
                q_sb.append(t)
            v_sb = []          # value [j, in] as 4 row tiles [128, 256]
            for jc in range(4):
                t = io.tile([P, D], f32, tag=f"v_sb{jc}", name=f"v_sb{jc}")
                nc.gpsimd.dma_start(out=t, in_=value[bass.ts(jc, P), :])
                v_sb.append(t)
            wq_sb = []         # Wq [in, d] as 2 in-chunk tiles [128, 256]
            wv_sb = []
            for ic in range(2):
                t = io.tile([P, D], f32, tag=f"wq_sb{ic}", name=f"wq_sb{ic}")
                nc.sync.dma_start(out=t, in_=wq[bass.ts(ic, P), :])
                wq_sb.append(t)
                t = io.tile([P, D], f32, tag=f"wv_sb{ic}", name=f"wv_sb{ic}")
                nc.sync.dma_start(out=t, in_=wv[bass.ts(ic, P), :])
                wv_sb.append(t)

            # ---------------- transposes of query / value ----------------
            # queryT [in, i]: 2 in-chunk tiles [128, 256]
            qT = [io.tile([P, 2 * P], f32, tag=f"qT{i}", name=f"qT{i}") for i in range(2)]
            for ic in range(2):
                for m in range(2):
                    pt = psum_t.tile([P, P], f32, tag="tr")
                    nc.tensor.transpose(pt, q_sb[m][:, bass.ts(ic, P)], ident)
                    nc.vector.tensor_copy(out=qT[ic][:, bass.ts(m, P)], in_=pt)
            # valueT [in, j]: 2 in-chunk tiles [128, 512]
            vT = [io.tile([P, TK], f32, tag=f"vT{i}", name=f"vT{i}") for i in range(2)]
            for ic in range(2):
                for jc in range(4):
                    pt = psum_t.tile([P, P], f32, tag="tr")
                    nc.tensor.transpose(pt, v_sb[jc][:, bass.ts(ic, P)], ident)
                    nc.vector.tensor_copy(out=vT[ic][:, bass.ts(jc, P)], in_=pt)

            # ---------------- projections (transposed layout) ----------------
            # Projections stay in PSUM; the base-map Sin ops read them there
            # (ScalarE is closer to PSUM) -- no SBUF staging copy.
            qp_ps = []
            for t in range(2):
                pp = psum_t.tile([P, TQ_C], f32, tag="proj", bufs=4, name=f"qp_ps{t}")
                for ic in range(2):
                    nc.tensor.matmul(pp, wq_sb[ic][:, bass.ts(t, P)], qT[ic],
                                     start=(ic == 0), stop=(ic == 1))
                qp_ps.append(pp)
            kp_ps = []
            for t in range(2):
                pp = psum_t.tile([P, TK], f32, tag="proj", bufs=4, name=f"kp_ps{t}")
                for ic in range(2):
                    nc.tensor.matmul(pp, wv_sb[ic][:, bass.ts(t, P)], vT[ic],
                                     start=(ic == 0), stop=(ic == 1))
                kp_ps.append(pp)

            # ---------------- base maps ----------------
            FQ, FK = 2 * TQ_C, 2 * TK     # free sizes of packed q/k tiles

            halfpi = consts.tile([P, 1], f32)
            nc.vector.memset(halfpi, HALF_PI)

            # Base maps via HALF-frequency sin/cos (ACT Sin args stay well
            # inside [-pi, pi] for any om1 <= 0.6), then angle-doubling:
            #   sin(om1 x) = 2 sh ch,  cos(om1 x) = 1 - 2 sh^2
            shq = qtmp.tile([P, FQ], f32, tag="tq", name="shq")
            chq = qtmp.tile([P, FQ], f32, tag="tq2", name="chq")
            for t in range(2):
                nc.scalar.activation(shq[:, bass.ts(t, TQ_C)], qp_ps[t], AF.Sin,
                                     scale=OM1 / 2)
                nc.scalar.activation(chq[:, bass.ts(t, TQ_C)], qp_ps[t], AF.Sin,
                                     bias=halfpi, scale=OM1 / 2)
            sq1 = qmaps.tile([P, FQ], f32, tag="sq", name="sq1")
            nc.vector.scalar_tensor_tensor(sq1, shq, 2.0, chq,
                                           op0=ALU.mult, op1=ALU.mult)
            uq1 = qtmp.tile([P, FQ], f32, tag="tq", name="uq1")
            nc.scalar.square(uq1, shq)
            cq1 = qmaps.tile([P, FQ], f32, tag="cq", name="cq1")
            nc.vector.tensor_scalar(cq1, uq1, -2.0, 1.0, op0=ALU.mult, op1=ALU.add)
            c2q = consts.tile([P, FQ], f32)
            nc.vector.tensor_scalar_mul(c2q, cq1, 2.0)

            shk = ktmp.tile([P, FK], f32, tag="tk", name="shk")
            chk = ktmp.tile([P, FK], f32, tag="tk2", name="chk")
            for t in range(2):
                nc.scalar.activation(shk[:, bass.ts(t, TK)], kp_ps[t], AF.Sin,
                                     scale=OM1 / 2)
                nc.scalar.activation(chk[:, bass.ts(t, TK)], kp_ps[t], AF.Sin,
                                     bias=halfpi, scale=OM1 / 2)
            sk1 = kmaps.tile([P, FK], f32, tag="sk", name="sk1")
            nc.vector.scalar_tensor_tensor(sk1, shk, 2.0, chk,
                                           op0=ALU.mult, op1=ALU.mult)
            uk1 = ktmp.tile([P, FK], f32, tag="tk", name="uk1")
            nc.scalar.square(uk1, shk)
            ck1 = kmaps.tile([P, FK], f32, tag="ck", name="ck1")
            nc.vector.tensor_scalar(ck1, uk1, -2.0, 1.0, op0=ALU.mult, op1=ALU.add)
            c2k = consts.tile([P, FK], f32)
            nc.vector.tensor_scalar_mul(c2k, ck1, 2.0)


            # ---------------- main loop: ladder + score matmuls ----------------
            score_ps = [psum_s.tile([P, TK], f32, tag="score", name=f"score{m}") for m in range(2)]

            mm_cnt = [0, 0]
            mm_total = R * 2 * 2   # per m-tile: R harmonics x 2 t-chunks x 2 products
            sqm = {1: sq1}
            cqm = {1: cq1}
            skm = {1: sk1}
            ckm = {1: ck1}
            for r in range(1, R + 1):
                if r == 1:
                    pass
                elif r % 2 == 0:
                    m2 = r // 2
                    # sin(2m) = (sin(m)*2)*cos(m) in one fused op
                    sq_r = qmaps.tile([P, FQ], f32, tag="sq", name=f"sq{r}")
                    nc.vector.scalar_tensor_tensor(sq_r, sqm[m2], 2.0, cqm[m2],
                                                   op0=ALU.mult, op1=ALU.mult)
                    # cos(2m) = 1 - 2*sin(m)^2 : Square on ScalarE + 1 tensor_scalar
                    u = qtmp.tile([P, FQ], f32, tag="tq", name=f"usq{r}")
                    nc.scalar.square(u, sqm[m2])
                    cq_r = qmaps.tile([P, FQ], f32, tag="cq", name=f"cq{r}")
                    nc.scalar.activation(cq_r, u, AF.Copy, bias=1.0, scale=-2.0)
                    sk_r = kmaps.tile([P, FK], f32, tag="sk", name=f"sk{r}")
                    nc.vector.scalar_tensor_tensor(sk_r, skm[m2], 2.0, ckm[m2],
                                                   op0=ALU.mult, op1=ALU.mult)
                    uk = ktmp.tile([P, FK], f32, tag="tk", name=f"usk{r}")
                    nc.scalar.square(uk, skm[m2])
                    ck_r = kmaps.tile([P, FK], f32, tag="ck", name=f"ck{r}")
                    nc.scalar.activation(ck_r, uk, AF.Copy, bias=1.0, scale=-2.0)
                    sqm[r], cqm[r], skm[r], ckm[r] = sq_r, cq_r, sk_r, ck_r
                else:
                    # odd r: Chebyshev recurrence s_r = 2c1*s_{r-1} - s_{r-2}
                    tq1 = qtmp.tile([P, FQ], f32, tag="tq", name=f"tq1_{r}")
                    nc.vector.tensor_mul(tq1, c2q, sqm[r - 1])
                    sq_r = qmaps.tile([P, FQ], f32, tag="sq", name=f"sq{r}")
                    nc.vector.tensor_sub(sq_r, tq1, sqm[r - 2])
                    tq2 = qtmp.tile([P, FQ], f32, tag="tq", name=f"tq2_{r}")
                    nc.vector.tensor_mul(tq2, c2q, cqm[r - 1])
                    cq_r = qmaps.tile([P, FQ], f32, tag="cq", name=f"cq{r}")
                    nc.vector.tensor_sub(cq_r, tq2, cqm[r - 2])
                    tk1 = ktmp.tile([P, FK], f32, tag="tk", name=f"tk1_{r}")
                    nc.vector.tensor_mul(tk1, c2k, skm[r - 1])
                    sk_r = kmaps.tile([P, FK], f32, tag="sk", name=f"sk{r}")
                    nc.vector.tensor_sub(sk_r, tk1, skm[r - 2])
                    tk2 = ktmp.tile([P, FK], f32, tag="tk", name=f"tk2_{r}")
                    nc.vector.tensor_mul(tk2, c2k, ckm[r - 1])
                    ck_r = kmaps.tile([P, FK], f32, tag="ck", name=f"ck{r}")
                    nc.vector.tensor_sub(ck_r, tk2, ckm[r - 2])
                    sqm[r], cqm[r], skm[r], ckm[r] = sq_r, cq_r, sk_r, ck_r

                # weighted q-side copies: awsq = (a_r s_d) * sin_r, awcq = (a_r s_d) * cos_r
                # odd-r casts ride on ScalarE (it has headroom); even-r on VectorE
                awsq = wmaps.tile([P, FQ], f16, tag="awsq", name=f"awsq{r}")
                awcq = wmaps.tile([P, FQ], f16, tag="awcq", name=f"awcq{r}")
                for t in range(2):
                    sl = bass.ts(t, TQ_C)
                    if r > 1 and (r % 2 == 1 or r <= 6):
                        nc.scalar.activation(awsq[:, sl], sqm[r][:, sl], AF.Copy,
                                             scale=wcols[r - 1][:, t:t+1])
                        nc.scalar.activation(awcq[:, sl], cqm[r][:, sl], AF.Copy,
                                             scale=wcols[r - 1][:, t:t+1])
                    else:
                        nc.vector.tensor_scalar_mul(awsq[:, sl], sqm[r][:, sl],
                                                    wcols[r - 1][:, t:t+1])
                        nc.vector.tensor_scalar_mul(awcq[:, sl], cqm[r][:, sl],
                                                    wcols[r - 1][:, t:t+1])

                ck16 = wmaps.tile([P, FK], f16, tag="ck16", name=f"ck16_{r}")
                nc.scalar.copy(out=ck16, in_=ckm[r])
                sk16 = wmaps.tile([P, FK], f16, tag="sk16", name=f"sk16_{r}")
                nc.scalar.copy(out=sk16, in_=skm[r])

                for m in range(2):
                    for t in range(2):
                        lhs_sl = bass.ds(t * TQ_C + m * P, P)
                        rhs_sl = bass.ts(t, TK)
                        nc.tensor.matmul(score_ps[m], awsq[:, lhs_sl],
                                         ck16[:, rhs_sl],
                                         start=(mm_cnt[m] == 0), stop=False,
                                         skip_group_check=True)
                        nc.tensor.matmul(score_ps[m], awcq[:, lhs_sl],
                                         sk16[:, rhs_sl],
                                         start=False,
                                         stop=(mm_cnt[m] == mm_total - 1),
                                         skip_group_check=True)
                        mm_cnt[m] += 1

            # ---------------- softmax ----------------
            # softmax without row-max: scores bounded well below fp32 exp
            # overflow (88.7); row-sum accumulates inside the Exp (accum_out).
            p_list, rinv_list = [], []
            for m in range(2):
                p_sb = soft.tile([P, TK], f32, tag="p_sb", name=f"p_sb{m}")
                rsum = soft.tile([P, 1], f32, tag="rsum")
                nc.scalar.activation(p_sb, score_ps[m], AF.Exp, accum_out=rsum)
                rinv = soft.tile([P, 1], f32, tag="rinv", name=f"rinv{m}")
                nc.vector.reciprocal(rinv, rsum)
                at = soft.tile([P, TK], f32, tag="attn_sb")
                nc.vector.tensor_scalar_mul(at, p_sb, rinv)
                nc.sync.dma_start(out=attn[bass.ts(m, P), :], in_=at)
                p_list.append(p_sb)
                rinv_list.append(rinv)

            # ---------------- out = attn @ value ----------------
            # transpose attn -> attnT [j, i] as 4 j-chunk tiles [128, 256]
            attnT = [soft.tile([P, TQ_C], f32, tag=f"attnT{i}", name=f"attnT{i}") for i in range(4)]
            for jc in range(4):
                for m in range(2):
                    pt = psum_t.tile([P, P], f32, tag="tr")
                    nc.tensor.transpose(pt, p_list[m][:, bass.ts(jc, P)], ident)
                    nc.vector.tensor_copy(out=attnT[jc][:, bass.ts(m, P)], in_=pt)
            for m in range(2):
                po = psum_t.tile([P, D], f32, tag="proj", bufs=4, name=f"out_ps{m}")
                for jc in range(4):
                    nc.tensor.matmul(po, attnT[jc][:, bass.ts(m, P)], v_sb[jc],
                                     start=(jc == 0), stop=(jc == 3))
                o_sb = soft.tile([P, D], f32, tag="o_sb")
                nc.vector.tensor_scalar_mul(o_sb, po, rinv_list[m])
                nc.gpsimd.dma_start(out=out[bass.ts(m, P), :], in_=o_sb)

    nc.compile()
    return nc


def _get_program(repeats=1):
    key = f"nc{repeats}"
    if key not in _CACHE:
        _CACHE[key] = _build_program(repeats)
    return _CACHE[key]


def kernel(query, value, Wq, Wv, scale):
    from concourse.bass_utils import run_bass_kernel_spmd

    nc = _get_program()
    query = np.ascontiguousarray(query, dtype=np.float32)
    value = np.ascontiguousarray(value, dtype=np.float32)
    Wq = np.ascontiguousarray(Wq, dtype=np.float32)
    Wv = np.ascontiguousarray(Wv, dtype=np.float32)
    scale = np.ascontiguousarray(scale, dtype=np.float32)

    in_maps = []
    for c in range(N_CORES):
        b, ih = c // 2, c % 2
        in_maps.append({
            "query": query[b, ih * TQ_C:(ih + 1) * TQ_C, :],
            "value": value[b],
            "wq": Wq, "wv": Wv, "scale": scale,
        })
    res = run_bass_kernel_spmd(nc, in_maps, list(range(N_CORES)))

    out = np.empty((B, TQ, D), dtype=np.float32)
    attn = np.empty((B, TQ, TK), dtype=np.float32)
    for c in range(N_CORES):
        b, ih = c // 2, c % 2
        out[b, ih * TQ_C:(ih + 1) * TQ_C, :] = res.results[c]["out"]
        attn[b, ih * TQ_C:(ih + 1) * TQ_C, :] = res.results[c]["attn"]
    return out, attn
